# revision 13
# baseline (speedup 1.0000x reference)
"""Trainium2 Bass kernel for nn_Attention_17016660426876.

Full-input contract: kernel(**inputs) takes the unsharded inputs and returns
the full (4, 2048, 1024) output. Internally: 8 NeuronCores, core c handles
batch b=c//2 and head-half hh=c%2 (8 of 16 heads). Head-tensor-parallel over
pairs of cores, data-parallel over batches. Each core emits a partial output
projection (contraction over its 512 head-features); host sums core pairs.
"""

import sys

sys.path.insert(0, "/opt/trn_rl_repo")

from contextlib import ExitStack

import numpy as np

import concourse.bass as bass
import concourse.mybir as mybir
import concourse.tile as tile
from concourse import bacc
from concourse.bass_utils import run_bass_kernel_spmd

B, N, C, H, D = 4, 2048, 1024, 16, 64
NCORES = 8
HL = H // 2          # heads per core
CL = HL * D          # 512 local head-features
F_QK = 2 * CL        # q+k local features
EPS = 1e-6
NQ = N // 512        # token quarters

F32 = mybir.dt.float32
F32R = mybir.dt.float32r
BF16 = mybir.dt.bfloat16

# dtype knobs
PV_DT = BF16         # storage dtype for P (exp output) and v (attn values)
QK_DT = BF16         # storage dtype for roped q/k (S-matmul operands)


def build_nc():
    nc = bacc.Bacc("TRN2", target_bir_lowering=False, debug=False,
                   num_devices=NCORES)

    xT = nc.dram_tensor("xT", [C, N], F32R, kind="ExternalInput")
    wq = nc.dram_tensor("wq", [8, 128, F_QK], F32R, kind="ExternalInput")
    wvT = nc.dram_tensor("wvT", [C, CL], F32R, kind="ExternalInput")
    wpT = nc.dram_tensor("wpT", [CL, C], F32R, kind="ExternalInput")
    tab_cq = nc.dram_tensor("tab_cq", [128, N], F32, kind="ExternalInput")
    tab_sq = nc.dram_tensor("tab_sq", [128, N], F32, kind="ExternalInput")
    tab_ck = nc.dram_tensor("tab_ck", [128, N], F32, kind="ExternalInput")
    tab_sk = nc.dram_tensor("tab_sk", [128, N], F32, kind="ExternalInput")
    rmat = nc.dram_tensor("rmat", [128, 128], F32R, kind="ExternalInput")
    blk = nc.dram_tensor("blk", [128, 128], F32R, kind="ExternalInput")
    out = nc.dram_tensor("out", [N, C], F32, kind="ExternalOutput")

    ACT = mybir.ActivationFunctionType

    with tile.TileContext(nc) as tc, ExitStack() as top:
        pers = top.enter_context(tc.tile_pool(name="pers", bufs=1))

        # long-lived SBUF
        qkT = [pers.tile([128, N], QK_DT, name=f"qkT{j}") for j in range(8)]
        # v with 64 appended ones-columns per head: (tokens, head, 0:64=v, 64:128=1)
        v5 = [pers.tile([128, HL * 128], PV_DT, name=f"v5_{t}") for t in range(16)]
        wvT_sb = [pers.tile([128, CL], F32R, name=f"wvT{i}") for i in range(8)]
        rmat_sb = pers.tile([128, 128], F32R, name="rmat_sb")
        blk_sb = pers.tile([128, 128], F32R, name="blk_sb")
        eps_sb = pers.tile([128, 1], F32, name="eps_sb")

        nc.vector.memset(eps_sb, EPS)
        nc.sync.dma_start(out=rmat_sb, in_=rmat[:, :])
        # PE warm-up: keep the HAM activity monitor busy while the big input
        # DMAs land, so real matmuls start at 2.4GHz instead of 1.2GHz
        with tc.tile_pool(name="warm", bufs=1, space="PSUM") as warmp:
            wps = warmp.tile([128, 128], F32, tag="warm", name="warm_ps")
            for i in range(40):
                nc.tensor.matmul(wps, rmat_sb, rmat_sb, start=True, stop=True)
        for i in range(8):
            nc.sync.dma_start(out=wvT_sb[i], in_=wvT[i * 128:(i + 1) * 128, :])
        nc.sync.dma_start(out=blk_sb, in_=blk[:, :])
        for t in range(16):
            nc.vector.memset(v5[t], 1.0)

        # ---------------- phase 1: qkv + rmsnorm + rope ----------------
        with ExitStack() as p1:
            xp = p1.enter_context(tc.tile_pool(name="xp", bufs=3))
            tabp = p1.enter_context(tc.tile_pool(name="tabp", bufs=2))
            wqp = p1.enter_context(tc.tile_pool(name="wqp", bufs=2))
            scr = p1.enter_context(tc.tile_pool(name="scr", bufs=3))
            pqk = p1.enter_context(tc.tile_pool(name="pqk", bufs=4, space="PSUM"))
            pv = p1.enter_context(tc.tile_pool(name="pv", bufs=2, space="PSUM"))
            pm = p1.enter_context(tc.tile_pool(name="pm", bufs=1, space="PSUM"))
            prot = p1.enter_context(tc.tile_pool(name="prot", bufs=1, space="PSUM"))

            for th in range(2):
                xq, tabq, tsq = [], [], []
                for qq in range(2):
                    tq = th * 2 + qq
                    ts = slice(tq * 512, tq * 512 + 512)
                    tsq.append(ts)
                    xts = []
                    for ci in range(8):
                        t = xp.tile([128, 512], F32R, tag=f"x{ci}",
                                    name=f"x{ci}_{tq}")
                        nc.sync.dma_start(out=t, in_=xT[ci * 128:(ci + 1) * 128, ts])
                        xts.append(t)
                    xq.append(xts)
                    tabs = {}
                    for nm, dr in (("cq", tab_cq), ("sq", tab_sq),
                                   ("ck", tab_ck), ("sk", tab_sk)):
                        t = tabp.tile([128, 512], F32, tag=nm, name=f"{nm}_{tq}")
                        nc.sync.dma_start(out=t, in_=dr[:, ts])
                        tabs[nm] = t
                    tabq.append(tabs)

                    # v: natural layout (tokens, features), ones at e=64..127
                    for tk in range(4):
                        tg = tq * 4 + tk
                        ps = pv.tile([128, CL], F32, tag="pv", name=f"pv{tg}")
                        for ci in range(8):
                            nc.tensor.matmul(
                                ps, (xts[ci][:, tk * 128:(tk + 1) * 128]),
                                (wvT_sb[ci]), start=(ci == 0), stop=(ci == 7))
                        nc.vector.tensor_copy(
                            out=v5[tg].rearrange("p (h e) -> p h e",
                                                 h=HL)[:, :, 0:64],
                            in_=ps.rearrange("p (h d) -> p h d", h=HL))

                # q,k: transposed layout (features, tokens); psum pair per j
                # shares each weight-stationary across both quarters
                for j in (0, 4, 1, 5, 2, 6, 3, 7):
                    wqt = wqp.tile([128, F_QK], F32R, tag="wq", name=f"wq{j}_{th}")
                    nc.sync.dma_start(out=wqt, in_=wq[j])
                    ps2 = [pqk.tile([128, 512], F32, tag="pqk",
                                    name=f"pqk{j}_{th}_{qq}") for qq in range(2)]
                    for ci in range(8):
                        for qq in range(2):
                            nc.tensor.matmul(
                                ps2[qq], (wqt[:, ci * 128:(ci + 1) * 128]),
                                (xq[qq][ci]), start=(ci == 0), stop=(ci == 7))
                    raws, sqs = [], []
                    for qq in range(2):
                        raw = scr.tile([128, 512], F32R, tag="raw",
                                       name=f"raw{j}_{th}_{qq}")
                        nc.vector.tensor_copy(out=raw, in_=ps2[qq])
                        raws.append(raw)
                        sq = scr.tile([128, 512], F32R, tag="sq",
                                      name=f"sq{j}_{th}_{qq}")
                        nc.gpsimd.tensor_mul(sq, raw.bitcast(F32), raw.bitcast(F32))
                        sqs.append(sq)
                    psms = []
                    for qq in range(2):
                        psm = pm.tile([128, 512], F32, tag="pm",
                                      name=f"pm{j}_{th}_{qq}")
                        nc.tensor.matmul(psm, (blk_sb), (sqs[qq]),
                                         start=True, stop=True)
                        psms.append(psm)
                    prs = []
                    for qq in range(2):
                        pr = prot.tile([128, 512], F32, tag="prot",
                                       name=f"prot{j}_{th}_{qq}")
                        nc.tensor.matmul(pr, (rmat_sb), (raws[qq]),
                                         start=True, stop=True)
                        prs.append(pr)
                    for qq in range(2):
                        rstd = scr.tile([128, 512], F32, tag="rstd",
                                        name=f"rstd{j}_{th}_{qq}")
                        nc.scalar.activation(rstd, psms[qq], ACT.Sqrt,
                                             bias=eps_sb, scale=1.0)
                        rb = scr.tile([128, 512], F32, tag="rb",
                                      name=f"rb{j}_{th}_{qq}")
                        nc.vector.reciprocal_approx_fast(out=rb, in_=rstd)
                        tabs = tabq[qq]
                        tc_, tssin = (tabs["cq"], tabs["sq"]) if j < 4 else \
                                     (tabs["ck"], tabs["sk"])
                        # y = (raw*cos + rot(raw)*sin) * rstd^-1
                        u = scr.tile([128, 512], F32, tag="u",
                                     name=f"u{j}_{th}_{qq}")
                        nc.gpsimd.tensor_mul(u, raws[qq].bitcast(F32), tc_)
                        w = scr.tile([128, 512], F32, tag="w",
                                     name=f"w{j}_{th}_{qq}")
                        nc.vector.tensor_mul(w, prs[qq], tssin)
                        v2 = scr.tile([128, 512], F32, tag="v2",
                                      name=f"v2{j}_{th}_{qq}")
                        nc.gpsimd.tensor_add(v2, u, w)
                        nc.vector.tensor_mul(qkT[j][:, tsq[qq]], v2, rb)

        tc.strict_bb_all_engine_barrier()

        # ---------------- phase 2: attention per head ----------------
        with ExitStack() as p2:
            wpp = p2.enter_context(tc.tile_pool(name="wpp", bufs=1))
            wpT_sb = [wpp.tile([128, C], F32R, name=f"wpT{i}") for i in range(4)]
            for i in range(4):
                nc.sync.dma_start(out=wpT_sb[i], in_=wpT[i * 128:(i + 1) * 128, :])
            aT = [wpp.tile([128, N], F32R, name=f"aT{i}") for i in range(4)]

            with ExitStack() as p2i:
                ptp = p2i.enter_context(tc.tile_pool(name="ptp", bufs=6))
                rbap = p2i.enter_context(tc.tile_pool(name="rbap", bufs=2))
                sps = p2i.enter_context(tc.tile_pool(name="sps", bufs=2, space="PSUM"))
                ops = p2i.enter_context(tc.tile_pool(name="ops", bufs=1, space="PSUM"))

                for hd in range(8):
                    j, row = hd // 2, (hd % 2) * 64
                    qt = qkT[j][row:row + 64, :]
                    kt = qkT[j + 4][row:row + 64, :]
                    op = ops.tile([128, N], F32, tag="op", name=f"op{hd}")
                    for kc in range(16):
                        vsl = v5[kc].rearrange("p (h e) -> p h e", h=HL)[:, hd, :]
                        spl = []
                        for qh in range(2):
                            sp = sps.tile([128, 1024], F32, tag="sp",
                                          name=f"sp{hd}_{kc}_{qh}")
                            for q2 in range(2):
                                qs = slice(qh * 1024 + q2 * 512,
                                           qh * 1024 + q2 * 512 + 512)
                                nc.tensor.matmul(
                                    sp[:, q2 * 512:q2 * 512 + 512],
                                    (kt[:, kc * 128:(kc + 1) * 128]),
                                    (qt[:, qs]), start=True, stop=True)
                            spl.append(sp)
                        ptl = []
                        for qh in range(2):
                            pt = ptp.tile([128, 1024], PV_DT, tag="pt",
                                          name=f"pt{hd}_{kc}_{qh}")
                            nc.scalar.activation(pt, spl[qh], ACT.Exp, scale=0.125)
                            ptl.append(pt)
                        for qh in range(2):
                            for q2 in range(2):
                                osl = slice(qh * 1024 + q2 * 512,
                                            qh * 1024 + q2 * 512 + 512)
                                nc.tensor.matmul(
                                    op[:, osl], vsl,
                                    ptl[qh][:, q2 * 512:q2 * 512 + 512],
                                    start=(kc == 0), stop=(kc == 15),
                                    skip_group_check=True)
                    anum = rbap.tile([64, N], F32, tag="anum", name=f"anum{hd}")
                    nc.vector.tensor_copy(out=anum, in_=op[0:64, :])
                    rden = rbap.tile([64, N], F32, tag="rden", name=f"rden{hd}")
                    nc.vector.tensor_copy(out=rden, in_=op[64:128, :])
                    rba = rbap.tile([64, N], F32, tag="rba", name=f"rba{hd}")
                    nc.vector.reciprocal_approx_fast(out=rba, in_=rden)
                    nc.vector.tensor_mul(aT[hd // 2][row:row + 64, :],
                                         anum, rba)

            # ---------------- phase 3: output projection ----------------
            with ExitStack() as p3:
                osb = p3.enter_context(tc.tile_pool(name="osb", bufs=3))
                ppj = p3.enter_context(tc.tile_pool(name="ppj", bufs=2, space="PSUM"))
                for tk in range(16):
                    pp = ppj.tile([128, C], F32, tag="pp", name=f"pp{tk}")
                    for ci in range(4):
                        for oh in range(2):
                            nc.tensor.matmul(
                                pp[:, oh * 512:oh * 512 + 512],
                                (aT[ci][:, tk * 128:(tk + 1) * 128]),
                                (wpT_sb[ci][:, oh * 512:oh * 512 + 512]),
                                start=(ci == 0), stop=(ci == 3),
                                skip_group_check=True)
                    ot = osb.tile([128, C], F32, tag="ot", name=f"ot{tk}")
                    nc.scalar.copy(out=ot, in_=pp)
                    nc.sync.dma_start(out=out[tk * 128:(tk + 1) * 128, :], in_=ot)

    nc.compile()
    return nc


def prep_inputs(x, cos, sin, w_qkv, w_proj, q_gamma, k_gamma):
    x = np.asarray(x, np.float32)
    cos = np.asarray(cos, np.float32)
    sin = np.asarray(sin, np.float32)
    w_qkv = np.asarray(w_qkv, np.float32)
    w_proj = np.asarray(w_proj, np.float32)
    q_gamma = np.asarray(q_gamma, np.float32)
    k_gamma = np.asarray(k_gamma, np.float32)

    cosT = np.ascontiguousarray(cos[0, 0].T)      # (64, N)
    sinT = np.ascontiguousarray(sin[0, 0].T)

    def tables(g):
        g_swap = g.reshape(D // 2, 2)[:, ::-1].reshape(D)
        ct = np.tile(cosT * g[:, None], (2, 1))
        st = np.tile(sinT * g_swap[:, None], (2, 1))
        return np.ascontiguousarray(ct), np.ascontiguousarray(st)

    cq_t, sq_t = tables(q_gamma)
    ck_t, sk_t = tables(k_gamma)

    rmat = np.zeros((128, 128), np.float32)
    idx = np.arange(0, 128, 2)
    rmat[idx, idx + 1] = 1.0
    rmat[idx + 1, idx] = -1.0

    blk = np.zeros((128, 128), np.float32)
    blk[:64, :64] = 1.0 / 64
    blk[64:, 64:] = 1.0 / 64

    in_maps = []
    for c in range(NCORES):
        b, hh = c // 2, c % 2
        xT = np.ascontiguousarray(x[b].T)
        wq_rows = w_qkv[512 * hh:512 * hh + 512]
        wk_rows = w_qkv[1024 + 512 * hh:1024 + 512 * hh + 512]
        wv_rows = w_qkv[2048 + 512 * hh:2048 + 512 * hh + 512]
        wqkT = np.concatenate([wq_rows, wk_rows], 0).T   # (1024 c, 1024 f)
        wq_tiled = np.ascontiguousarray(
            wqkT.reshape(8, 128, 8, 128).transpose(2, 1, 0, 3).reshape(8, 128, F_QK))
        wvT = np.ascontiguousarray(wv_rows.T)            # (1024, 512)
        wpT = np.ascontiguousarray(w_proj[:, 512 * hh:512 * hh + 512].T)
        in_maps.append({
            "xT": xT, "wq": wq_tiled, "wvT": wvT, "wpT": wpT,
            "tab_cq": cq_t, "tab_sq": sq_t, "tab_ck": ck_t, "tab_sk": sk_t,
            "rmat": rmat, "blk": blk,
        })
    return in_maps


_NC_CACHE = None


def get_nc():
    global _NC_CACHE
    if _NC_CACHE is None:
        _NC_CACHE = build_nc()
    return _NC_CACHE


def kernel(x, cos, sin, w_qkv, w_proj, q_gamma, k_gamma):
    nc = get_nc()
    in_maps = prep_inputs(x, cos, sin, w_qkv, w_proj, q_gamma, k_gamma)
    res = run_bass_kernel_spmd(nc, in_maps, list(range(NCORES)))
    parts = [res.results[c]["out"] for c in range(NCORES)]
    out = np.stack([parts[2 * b] + parts[2 * b + 1] for b in range(B)])
    return out.astype(np.float32)
